# revision 38
# baseline (speedup 1.0000x reference)
"""Multi-head attention (QKV proj + RoPE + causal SDPA + out proj) on 8 TRN2 cores.

Sharding: core c = 4*b + g handles batch b (of 2) and head-group g (of 4,
i.e. 4 heads = 512 feature dims): column-sharded w_q/w_kv, row-sharded w_o;
the host sums the 4 per-group out partials per batch (the w_o all-reduce).

All inputs are host-cast to bf16 and host-pre-arranged into the exact
partition-major SBUF layouts, then DMA'd directly into place -- no on-device
staging or dtype-conversion copies. The output is written bf16 (partials are
summed in f32 on the host). Device dataflow is feature-major ([d, s] q/k,
[s, d] v) so no on-device transposes are needed; causality is exploited at
128-block granularity with a triangle mask on the diagonal blocks only.
Attention runs a one-block software pipeline: the S matmul of block i+1
issues on the PE before the PV of block i, hiding the exp latency. The
softmax denominator is built by a bf16 add-tree over the per-block exp tiles
on the DVE (plus one 512-row matmul per (head, chunk)) instead of per-block
row-sum matmuls; its normalization tail is deferred one round so the PE
never waits on the tree. Because the PE queue executes in order, the QKV
projection of chunk j+1 (and at the last chunk, the out-projection of the
already-normalized rows) is emitted as 12 independent chains interleaved
into chunk j's attention, so exp-bound stretches never stall the PE. The
exp tiles live in two persistent 16-slot buffers zeroed once up front
(diagonal zero strips survive reuse), and out-projection PSUM evacuation
alternates ACT/DVE.
"""

import sys

import numpy as np

sys.path.insert(0, "/opt/trn_rl_repo")

EMB = 2048
SEQ = 2048
N_HEAD = 16
HD = 128
BATCH = 2
N_CORES = 8
GROUPS = 4
HPG = N_HEAD // GROUPS  # 4 heads per group
DPG = HPG * HD  # 512 feature dims per group
NE = EMB // 128  # 16 e-blocks
SCALE = float(HD) ** -0.5


def _host_tables(seq):
    """cos / sign-folded sin RoPE tables in [d, s] layout + triangle mask."""
    d = HD
    inv = 1.0 / (10000.0 ** (np.arange(0, d, 2, dtype=np.float64) / d))
    pos = np.arange(seq, dtype=np.float64)[None, :] * inv[:, None]
    ang = np.concatenate([pos, pos], axis=0)  # [128, s]
    cos_t = np.cos(ang)
    sin = np.sin(ang)
    sinm = np.concatenate([-sin[:64], sin[64:]], axis=0)
    ko = np.arange(128)[:, None]
    qo = np.arange(128)[None, :]
    mask_t = (qo >= ko).astype(np.float64)
    return cos_t, sinm, mask_t


def build(seq=SEQ, has_bias=False, reps=1):
    import concourse.bacc as bacc
    import concourse.tile as tile
    from concourse import mybir

    f32 = mybir.dt.float32
    bf16 = mybir.dt.bfloat16

    assert seq % 512 == 0
    nj = seq // 512

    nc = bacc.Bacc("TRN2", target_bir_lowering=False, debug=False,
                   num_devices=N_CORES, name=f"mha8v9r{reps}")

    # all host-pre-arranged, partition-major, bf16
    xt_d = nc.dram_tensor("xt", [128, NE, seq], bf16, kind="ExternalInput")
    wq_d = nc.dram_tensor("wq", [128, NE, DPG], bf16, kind="ExternalInput")
    wk_d = nc.dram_tensor("wk", [128, NE, DPG], bf16, kind="ExternalInput")
    wv_d = nc.dram_tensor("wv", [128, NE, DPG], bf16, kind="ExternalInput")
    wo_d = nc.dram_tensor("wo", [128, HPG, EMB], bf16, kind="ExternalInput")
    bo_d = nc.dram_tensor("bo", [1, EMB], f32, kind="ExternalInput")
    cos_d = nc.dram_tensor("cosT", [HD, seq], bf16, kind="ExternalInput")
    sinm_d = nc.dram_tensor("sinM", [HD, seq], bf16, kind="ExternalInput")
    mask_d = nc.dram_tensor("maskT", [128, 128], bf16, kind="ExternalInput")
    out_d = nc.dram_tensor("out", [seq, EMB], bf16, kind="ExternalOutput")

    with tile.TileContext(nc) as tc:
        for _ in range(reps):
            _emit(nc, tc, tile, mybir, seq, nj,
                  xt_d, wq_d, wk_d, wv_d, wo_d, bo_d, cos_d, sinm_d, mask_d,
                  out_d, has_bias)
    nc.compile()
    return nc


def _emit(nc, tc, tile, mybir, seq, nj,
          xt_d, wq_d, wk_d, wv_d, wo_d, bo_d, cos_d, sinm_d, mask_d, out_d,
          has_bias):
    from contextlib import ExitStack

    f32 = mybir.dt.float32
    bf16 = mybir.dt.bfloat16
    EXP = mybir.ActivationFunctionType.Exp
    nsb = seq // 128

    ctx = ExitStack()
    with ctx:
        persist = ctx.enter_context(tc.tile_pool(name="persist", bufs=1))

        # right-side, freed before out-projection: tables + projection weights
        ph2 = ExitStack()
        ph2_pool = ph2.enter_context(tc.tile_pool(name="ph2", bufs=1, side="right"))
        xt_pool = ph2.enter_context(tc.tile_pool(name="xt", bufs=2, side="right"))

        # ---- constants ----
        mask_sb = persist.tile([128, 128], bf16, name="mask_sb")
        cos_sb = ph2_pool.tile([128, seq], bf16, name="cos_sb")
        sinm_sb = ph2_pool.tile([128, seq], bf16, name="sinm_sb")

        def load_tables():
            nc.scalar.dma_start(mask_sb, mask_d[:])
            for half in range(2):
                hs = slice(half * (seq // 2), (half + 1) * (seq // 2))
                eng = nc.scalar if half else nc.sync
                eng.dma_start(cos_sb[:, hs], cos_d[:, hs])
                eng2 = nc.sync if half else nc.scalar
                eng2.dma_start(sinm_sb[:, hs], sinm_d[:, hs])

        # persistent activations
        kt = persist.tile([128, HPG, seq], bf16, name="kt")    # [d, h, s]
        yt = persist.tile([128, HPG, seq], bf16, name="yt")
        v_sb = persist.tile([128, nsb, DPG], bf16, name="v_sb")  # [s_in, blk, d]

        w_sb = {}
        for nm in ("wq", "wk", "wv"):
            w_sb[nm] = ph2_pool.tile([128, NE, DPG], bf16, name=f"{nm}_sb")

        def load_w(nm, wd, e2, eng):
            eng.dma_start(w_sb[nm][:, 2 * e2:2 * e2 + 2, :],
                          wd[:, 2 * e2:2 * e2 + 2, :])

        def load_xt(xt_j, j, e2):
            eng = nc.sync if e2 % 2 == 0 else nc.scalar
            eng.dma_start(xt_j[:, 2 * e2:2 * e2 + 2, :],
                          xt_d[:, 2 * e2:2 * e2 + 2, j * 512:(j + 1) * 512])

        qtj_pool = ctx.enter_context(tc.tile_pool(name="qtj", bufs=2))
        rope_pool = ctx.enter_context(tc.tile_pool(name="rope", bufs=4))
        sm_pool = ctx.enter_context(tc.tile_pool(name="sm", bufs=2))
        out_pool = ctx.enter_context(tc.tile_pool(name="outp", bufs=4))

        # manual double-buffered exp-block store; zeroed once up front so the
        # diagonal slots' zero strips are in place (they survive round reuse:
        # exp rewrites [qoff:512] and the in-place add-tree only ever adds
        # zeros onto a strip)
        pts = [persist.tile([128, 16, 512], bf16, name=f"pts{b}")
               for b in range(2)]
        for b in range(2):
            nc.vector.memset(pts[b][:], 0.0)

        ones_col = persist.tile([128, 1], bf16, name="ones_col")
        nc.vector.memset(ones_col, 1.0)

        ps_ctx = ExitStack()
        ps2 = ps_ctx.enter_context(tc.tile_pool(name="ps2", bufs=2, space="PSUM"))
        ps1_ctx = ExitStack()
        ps1 = ps1_ctx.enter_context(tc.tile_pool(name="ps1", bufs=2, space="PSUM"))

        def rope(dst, h, j, proj_ps):
            sl = slice(j * 512, (j + 1) * 512)
            qs = rope_pool.tile([128, 512], bf16, name=f"qs_{h}_{j}", tag="qs")
            nc.vector.tensor_copy(qs, proj_ps)  # PSUM f32 -> SBUF bf16
            rot = rope_pool.tile([128, 512], bf16, name=f"rot_{h}_{j}", tag="rot")
            nc.vector.tensor_copy(rot[0:64, :], qs[64:128, :])
            nc.vector.tensor_copy(rot[64:128, :], qs[0:64, :])
            nc.vector.tensor_mul(qs, qs, cos_sb[:, sl])
            nc.vector.tensor_mul(rot, rot, sinm_sb[:, sl])
            nc.vector.tensor_add(dst, qs, rot)

        wo_pool = None
        wo_sb = None
        bo_sb = ones_row = None
        state = {"ps3": None, "tail": None}

        def make_tail(h, j, pt, ot):
            # row-sum of the tree root + normalize; deferred one round so the
            # PE hits the rs matmul only after the DVE tree surely finished
            def tail():
                rs = ps2.tile([1, 512], f32, name=f"rs_{h}_{j}", tag="rs",
                              bufs=1)
                nc.tensor.matmul(rs, ones_col, pt[:, 0, :], start=True,
                                 stop=True)
                rsf = sm_pool.tile([1, 512], f32, name=f"rsf_{h}_{j}",
                                   tag="rsf")
                nc.vector.tensor_copy(rsf, rs)
                nc.vector.reciprocal_approx_fast(rsf, rsf)
                rb = sm_pool.tile([128, 512], f32, name=f"rb_{h}_{j}",
                                  tag="rb")
                nc.gpsimd.partition_broadcast(rb, rsf)
                nc.vector.tensor_mul(yt[:, h, j * 512:(j + 1) * 512], ot, rb)
            return tail

        def load_xt_round(j):
            xt_j = xt_pool.tile([128, NE, 512], bf16, name=f"xt_{j}", tag="xt")
            for e2 in range(NE // 2):
                load_xt(xt_j, j, e2)
                if j == 0:
                    load_w("wq", wq_d, e2, nc.scalar)
                    load_w("wk", wk_d, e2, nc.sync)
            if j == 0:
                # tables before wv: the first rope (gates kt and all round-0
                # attention) needs cos/sin earlier than v_proj needs wv
                load_tables()
                for e2 in range(NE // 2):
                    load_w("wv", wv_d, e2, nc.scalar if e2 % 2 else nc.sync)
            return xt_j

        def proj_chains(j, xt_j):
            """QKV projection for chunk j as 12 independent PE chains (~3.4us
            each) that get interleaved into the previous round's attention so
            the in-order PE queue always has exp-independent work."""
            qt_j = qtj_pool.tile([128, HPG, 512], bf16, name=f"qt_{j}",
                                 tag="qtj")
            chains = []
            for h in range(HPG):
                for nm in ("wq", "wk"):
                    def qk_chain(h=h, nm=nm, j=j):
                        pp = ps1.tile([128, 512], f32, name=f"pp_{nm}_{h}_{j}",
                                      tag="proj")
                        for e in range(NE):
                            nc.tensor.matmul(
                                pp, w_sb[nm][:, e, h * 128:(h + 1) * 128],
                                xt_j[:, e, :], start=(e == 0),
                                stop=(e == NE - 1))
                        if nm == "wq":
                            rope(qt_j[:, h, :], h, j, pp)
                        else:
                            rope(kt[:, h, j * 512:(j + 1) * 512], h, j, pp)
                    chains.append(qk_chain)
            for sb in range(4):
                def v_chain(sb=sb, j=j):
                    i_blk = j * 4 + sb
                    vp = ps1.tile([128, DPG], f32, name=f"vp_{i_blk}",
                                  tag="proj")
                    for e in range(NE):
                        nc.tensor.matmul(
                            vp, xt_j[:, e, sb * 128:(sb + 1) * 128],
                            w_sb["wv"][:, e, :], start=(e == 0),
                            stop=(e == NE - 1))
                    nc.vector.tensor_copy(v_sb[:, i_blk, :], vp)
                chains.append(v_chain)
            return qt_j, chains

        def outproj_chain(sb, ec2):
            def chain():
                ssl = slice(sb * 128, (sb + 1) * 128)
                ob = out_pool.tile([128, 1024], bf16, name=f"ob_{sb}_{ec2}",
                                   tag="ob")
                for half in range(2):
                    ec = 2 * ec2 + half
                    esl = slice(ec * 512, (ec + 1) * 512)
                    op = state["ps3"].tile([128, 512], f32,
                                           name=f"op_{sb}_{ec}", tag="op")
                    for h in range(HPG):
                        nc.tensor.matmul(op, yt[:, h, ssl], wo_sb[:, h, esl],
                                         start=(h == 0),
                                         stop=(not has_bias and h == HPG - 1))
                    if has_bias:
                        nc.tensor.matmul(op, ones_row, bo_sb[:, esl],
                                         start=False, stop=True)
                    # alternate PSUM evacuation between ACT and DVE
                    if half == 0:
                        nc.scalar.copy(ob[:, 0:512], op)
                    else:
                        nc.vector.tensor_copy(ob[:, 512:1024], op)
                eng = nc.sync if (sb + ec2) % 2 == 0 else nc.scalar
                eng.dma_start(out_d[slice(sb * 128, (sb + 1) * 128),
                                    ec2 * 1024:(ec2 + 1) * 1024], ob)
            return chain

        def attn_head(h, j, qt_j, chainq, interleave_within):
            nblk = 4 * j + 4
            ot = ps2.tile([128, 512], f32, name=f"ot_{h}_{j}", tag="ot")
            # pt (pts ring): one slot per k-block, q-aligned; the denominator is
            # a bf16 add-tree over slots (split DVE/Pool) instead of per-block
            # row-sum matmuls on the PE.
            pt = pts[(j * HPG + h) % 2]

            def emit_s(i):
                # S matmul + exp (+ triangle mask on diagonal blocks)
                m = i - 4 * j
                qoff = max(m, 0) * 128
                n = 512 - qoff
                st_ps = ps2.tile([128, 512], f32, name=f"st_{h}_{j}_{i}",
                                 tag="st", bufs=3)
                nc.tensor.matmul(
                    st_ps[:, 0:n], kt[:, h, i * 128:(i + 1) * 128],
                    qt_j[:, h, qoff:512], start=True, stop=True)
                nc.scalar.activation(pt[:, i, qoff:512], st_ps[:, 0:n], EXP,
                                     scale=SCALE)
                if m >= 0:
                    nc.vector.tensor_mul(pt[:, i, qoff:qoff + 128],
                                         pt[:, i, qoff:qoff + 128], mask_sb)
                return i

            def emit_pv(i):
                qoff = max(i - 4 * j, 0) * 128
                nc.tensor.matmul(ot[:, qoff:512],
                                 v_sb[:, i, h * 128:(h + 1) * 128],
                                 pt[:, i, qoff:512],
                                 start=(i == 0), stop=(i == nblk - 1))

            # one-block software pipeline: S(i+1) issues on the PE before
            # PV(i), so the PE isn't waiting on exp(i); the previous round's
            # deferred row-sum matmul slots in after S(1).
            prev = None
            for i in range(nblk):
                cur = emit_s(i)
                if i == 1 and state["tail"] is not None:
                    state["tail"]()
                    state["tail"] = None
                if prev is not None:
                    emit_pv(prev)
                if interleave_within and i in (7, 13) and chainq:
                    chainq.pop(0)()
                prev = cur
            emit_pv(prev)
            # denominator add-tree on the DVE (in place over slots; slot 0
            # ends up holding the sum of all exp blocks)
            s = 1
            while s < nblk:
                for m2 in range(0, nblk - s, 2 * s):
                    nc.vector.tensor_add(pt[:, m2, :], pt[:, m2, :],
                                         pt[:, m2 + s, :])
                s *= 2
            state["tail"] = make_tail(h, j, pt, ot)

        # ---- prologue: chunk-0 loads + projections ----
        xt_j = load_xt_round(0)
        qt_cur, chains0 = proj_chains(0, xt_j)
        for c in chains0:
            c()

        late_chains = []
        for j in range(nj):
            if j + 1 < nj:
                xt_n = load_xt_round(j + 1)
                qt_next, chainq = proj_chains(j + 1, xt_n)
            else:
                # projections done with xT/w/tables: free the right side and
                # DMA the out-projection weights; overlaps round-3 attention,
                # which interleaves the out-projection chains whose yt rows
                # (q-chunks 0-2) are already normalized.
                ps1_ctx.close()
                ph2.close()
                wo_pool = ctx.enter_context(tc.tile_pool(name="wop", bufs=1))
                wo_sb = wo_pool.tile([128, HPG, EMB], bf16, name="wo_sb")
                if has_bias:
                    ones_row = wo_pool.tile([1, 128], bf16, name="ones_row")
                    nc.vector.memset(ones_row, 1.0)
                    bo_f32 = wo_pool.tile([1, EMB], f32, name="bo_f32")
                    nc.scalar.dma_start(bo_f32, bo_d[:])
                    bo_sb = wo_pool.tile([1, EMB], bf16, name="bo_sb")
                    nc.gpsimd.tensor_copy(bo_sb, bo_f32)
                for h in range(HPG):
                    eng = nc.scalar if h % 2 else nc.sync
                    eng.dma_start(wo_sb[:, h, :], wo_d[:, h, :])
                state["ps3"] = ps_ctx.enter_context(
                    tc.tile_pool(name="ps3", bufs=2, space="PSUM"))
                qt_next = None
                chainq = [outproj_chain(sb, ec2) for sb in range(nsb - 4)
                          for ec2 in range(EMB // 1024)]
                late_chains = [outproj_chain(sb, ec2)
                               for sb in range(nsb - 4, nsb)
                               for ec2 in range(EMB // 1024)]

            for h in range(HPG):
                # at j==3 head 0, hold chains back until wo_sb has landed
                attn_head(h, j, qt_cur, chainq,
                          interleave_within=(j < nj - 1 or h > 0))
                slots_left = HPG - 1 - h
                take = -(-len(chainq) // (slots_left + 1))  # ceil
                for _ in range(min(take, len(chainq))):
                    chainq.pop(0)()
            while chainq:
                chainq.pop(0)()
            qt_cur = qt_next

        state["tail"]()
        state["tail"] = None
        for c in late_chains:
            c()
        ps_ctx.close()


_NC_CACHE = {}


def _get_nc(seq=SEQ, has_bias=False, reps=1):
    key = (seq, has_bias, reps)
    if key not in _NC_CACHE:
        _NC_CACHE[key] = build(seq, has_bias, reps)
    return _NC_CACHE[key]


def make_in_maps(x, w_kv, w_q, w_o, b_o, seq=SEQ):
    """Shard full inputs into the 8 per-core input dicts (host-side prep)."""
    import ml_dtypes

    bf = ml_dtypes.bfloat16
    cos_t, sinm, mask_t = _host_tables(seq)
    cos_t = np.ascontiguousarray(cos_t.astype(bf))
    sinm = np.ascontiguousarray(sinm.astype(bf))
    mask_t = np.ascontiguousarray(mask_t.astype(bf))
    zeros_bo = np.zeros((1, EMB), np.float32)

    def pmaj(a, nblk, width):
        # [nblk*128, width] -> [128, nblk, width] partition-major
        return np.ascontiguousarray(
            np.asarray(a).reshape(nblk, 128, width).transpose(1, 0, 2).astype(bf))

    xts = [pmaj(np.asarray(x[b], np.float32).T, NE, seq) for b in range(BATCH)]
    in_maps = []
    for c in range(N_CORES):
        b, g = divmod(c, GROUPS)
        d0 = g * DPG
        in_maps.append({
            "xt": xts[b],
            "wq": pmaj(w_q[:, d0:d0 + DPG], NE, DPG),
            "wk": pmaj(w_kv[:, d0:d0 + DPG], NE, DPG),
            "wv": pmaj(w_kv[:, EMB + d0:EMB + d0 + DPG], NE, DPG),
            "wo": pmaj(w_o[d0:d0 + DPG, :], HPG, EMB),
            "bo": (np.ascontiguousarray(b_o.reshape(1, EMB), np.float32)
                   if g == 0 else zeros_bo),
            "cosT": cos_t,
            "sinM": sinm,
            "maskT": mask_t,
        })
    return in_maps


def kernel(x, w_kv, w_q, w_o, b_o):
    from concourse.bass_utils import run_bass_kernel_spmd

    x = np.asarray(x, np.float32)
    nc = _get_nc(SEQ, has_bias=bool(np.any(np.asarray(b_o))))
    in_maps = make_in_maps(x, np.asarray(w_kv, np.float32),
                           np.asarray(w_q, np.float32),
                           np.asarray(w_o, np.float32),
                           np.asarray(b_o, np.float32), SEQ)
    res = run_bass_kernel_spmd(nc, in_maps, core_ids=list(range(N_CORES)))
    parts = [res.results[c]["out"].astype(np.float32) for c in range(N_CORES)]
    out = np.stack(
        [parts[0] + parts[1] + parts[2] + parts[3],
         parts[4] + parts[5] + parts[6] + parts[7]], axis=0)
    return out.astype(np.float32)

